# revision 26
# baseline (speedup 1.0000x reference)
"""MultiHeadAttention Trainium2 kernel.

Problem: B=8, S=1024, D=768, H=12, DH=64, fp32.
Sharding: batch across the 8 cores (data parallel) — each core computes
attention for one batch element with the full weights.

Per-core algorithm (transposed-attention layout, no attn transposes):
  xT[o] = x^T chunks             [128(d), 1024(s)]  (PE transpose of x tiles)
  QT2/KT2 per head-pair          [128(2*64e), 1024(s)] = W_pair^T-matmul + bias
  V natural (all heads upfront)  packed [128(t), 8(tc), 12(h), 66] w/ ones col
  scoresT per (head, t-chunk)    [128(t), 1024(s)] = KT_slice.T @ QT
  attnT = Exp(scoresT/8)         (ScalarE, no max subtraction: scores ~ N(0,1))
  outT += [V|1].T @ attnT        [65, 1024]: row 64 = softmax denominators
  normalize: outT[0:64] * (1/den) broadcast (DVE recip + PE ones-matmul bcast)
  final = outT_all.T-matmul WoT + (bo + bv_cat @ Wo^T)

All matmuls run as float32r (fp32 storage, ~3 cols/cycle at N>=256; ~+61ns
when the stationary operand changes, so loops are ordered to reuse lhsT).
The two heads of a pair are interleaved so their K=64 scores matmuls can run
concurrently in separate PE row groups (base_partition 0 / 64).
"""

import numpy as np

import concourse.bass as bass
import concourse.tile as tile
from concourse import bacc, mybir
from concourse.bass_utils import run_bass_kernel_spmd
from concourse.masks import make_identity

B, S, D, H = 8, 1024, 768, 12
DH = D // H  # 64
P = 128
NO = D // P  # 6 d-chunks
NS = S // P  # 8 s/t-chunks
NPAIR = H // 2  # 6 head pairs
F32 = mybir.dt.float32
F32R = mybir.dt.float32r
BF16 = mybir.dt.bfloat16

N_CORES = 8


def r(ap):
    """Bitcast a matmul operand to float32r (full-rate fp32 matmul mode)."""
    return ap.bitcast(F32R)


def build_nc(loop_iters=None, phases=("setup", "v", "qkv", "attn", "wo")):
    nc = bacc.Bacc("TRN2", target_bir_lowering=False, debug=False)

    x_h = nc.declare_dram_parameter("x", [S, D], F32, isOutput=False)
    wq_h = nc.declare_dram_parameter("Wq", [H, D, DH], F32, isOutput=False)
    bq_h = nc.declare_dram_parameter("bq", [H, DH], F32, isOutput=False)
    wk_h = nc.declare_dram_parameter("Wk", [H, D, DH], F32, isOutput=False)
    bk_h = nc.declare_dram_parameter("bk", [H, DH], F32, isOutput=False)
    wv_h = nc.declare_dram_parameter("Wv", [H, D, DH], F32, isOutput=False)
    bv_h = nc.declare_dram_parameter("bv", [H, DH], F32, isOutput=False)
    wo_h = nc.declare_dram_parameter("Wo", [D, D], F32, isOutput=False)
    bo_h = nc.declare_dram_parameter("bo", [D], F32, isOutput=False)
    out_h = nc.declare_dram_parameter("out", [S, D], F32, isOutput=True)

    aps = dict(
        x=x_h.ap(), Wq=wq_h.ap(), bq=bq_h.ap(), Wk=wk_h.ap(), bk=bk_h.ap(),
        Wv=wv_h.ap(), bv=bv_h.ap(), Wo=wo_h.ap(), bo=bo_h.ap(), out=out_h.ap(),
    )

    from contextlib import ExitStack

    with tile.TileContext(nc) as tc:
        with nc.allow_low_precision("fp32r rounding of PE matmul operands"):
            with ExitStack() as ctx:
                pools = make_pools(tc, ctx)
                if loop_iters is None:
                    build_body(nc, tc, pools, aps, phases)
                else:
                    with tc.For_i(0, loop_iters, 1):
                        build_body(nc, tc, pools, aps, phases)
    nc.compile()
    return nc


def make_pools(tc, ctx):
    pools = {}
    pools["singles"] = ctx.enter_context(tc.tile_pool(name="singles", bufs=1))
    # PSUM: sc 2x[128,1024] (4 banks) + outT 2x[65,1024] (4 banks) = 8 banks.
    pools["sc_psum"] = ctx.enter_context(
        tc.tile_pool(name="sc_psum", bufs=3, space="PSUM")
    )
    pools["out_psum"] = ctx.enter_context(
        tc.tile_pool(name="out_psum", bufs=1, space="PSUM")
    )
    pools["ldpool"] = ctx.enter_context(tc.tile_pool(name="ldpool", bufs=2))
    pools["qkpool"] = ctx.enter_context(tc.tile_pool(name="qkpool", bufs=2))
    pools["wpool"] = ctx.enter_context(tc.tile_pool(name="wpool", bufs=2))
    pools["attnpool"] = ctx.enter_context(tc.tile_pool(name="attnpool", bufs=5))
    pools["rpool"] = ctx.enter_context(tc.tile_pool(name="rpool", bufs=2))
    pools["fpool"] = ctx.enter_context(tc.tile_pool(name="fpool", bufs=2))
    return pools


def build_body(nc, tc, pools, aps, phases=("setup", "v", "qkv", "attn", "wo")):
    singles = pools["singles"]
    sc_psum = pools["sc_psum"]
    out_psum = pools["out_psum"]
    x_d, out_d = aps["x"], aps["out"]
    wq_d, wk_d, wv_d, wo_d = aps["Wq"], aps["Wk"], aps["Wv"], aps["Wo"]
    bq_d, bk_d, bv_d, bo_d = aps["bq"], aps["bk"], aps["bv"], aps["bo"]

    identity = singles.tile([P, P], F32, tag="identity", name="identity")
    make_identity(nc, identity)

    # xT chunks: xT[o][d, s] = x[s, o*128+d]
    xT = [singles.tile([P, S], F32, tag=f"xT{o}", name=f"xT{o}") for o in range(NO)]
    # WoT chunks: woT[o][d, j] = Wo[j, o*128+d]
    woT = [singles.tile([P, D], F32, tag=f"woT{o}", name=f"woT{o}") for o in range(NO)]
    # normalized outT stacked chunks: outTall[p][i*64+e, s] = head(2p+i) out^T
    outTall = [
        singles.tile([P, S], F32, tag=f"outTall{p}", name=f"outTall{p}")
        for p in range(NPAIR)
    ]
    # packed V: [t-part, t-chunk, head, 66]; col 64 = 1.0 (denominator trick)
    vall = singles.tile([P, NS, H, DH + 2], F32, tag="vall", name="vall")

    bq_pairs = singles.tile([P, NPAIR], F32, tag="bq_pairs", name="bq_pairs")
    bk_pairs = singles.tile([P, NPAIR], F32, tag="bk_pairs", name="bk_pairs")
    bv_col = singles.tile([P, NO], F32, tag="bv_col", name="bv_col")
    bo1 = singles.tile([1, D], F32, tag="bo1", name="bo1")
    boB = singles.tile([P, D], F32, tag="boB", name="boB")
    ones_f32 = singles.tile([P, NS], F32, tag="ones_f32", name="ones_f32")
    nc.vector.memset(ones_f32, 1.0)
    zeros_f32 = singles.tile([P, 1], F32, tag="zeros_f32", name="zeros_f32")
    nc.vector.memset(zeros_f32, 0.0)
    # ones column of packed V (denominator trick)
    nc.vector.tensor_copy(
        out=r(vall[:, :, :, DH : DH + 1]),
        in_=ones_f32[:, 0:1, None].to_broadcast((P, NS, H, 1)),
    )

    # ---- setup: load + PE-transpose x and Wo (batched copies, 3 blocks) ----
    ldpool = pools["ldpool"]
    for s in range(NS if "setup" in phases else 0):
        x_sb = ldpool.tile([P, D], F32, tag="x_nat", name="x_nat")
        nc.sync.dma_start(out=x_sb, in_=x_d[s * P : (s + 1) * P, :])
        for og in range(2):  # groups of 3 chunks -> [128, 384] psum
            ps = sc_psum.tile([P, S], F32, tag="sc", name="sc")
            for j in range(3):
                o = og * 3 + j
                nc.tensor.transpose(
                    ps[:, j * P : (j + 1) * P],
                    x_sb[:, o * P : (o + 1) * P],
                    identity,
                )
            for j in range(3):
                o = og * 3 + j
                nc.vector.tensor_copy(
                    out=r(xT[o][:, s * P : (s + 1) * P]),
                    in_=ps[:, j * P : (j + 1) * P],
                )
    for jt in range(NO if "setup" in phases else 0):
        wo_sb = ldpool.tile([P, D], F32, tag="wo_nat", name="wo_nat")
        nc.sync.dma_start(out=wo_sb, in_=wo_d[jt * P : (jt + 1) * P, :])
        for og in range(2):
            ps = sc_psum.tile([P, S], F32, tag="sc", name="sc")
            for j in range(3):
                o = og * 3 + j
                nc.tensor.transpose(
                    ps[:, j * P : (j + 1) * P],
                    wo_sb[:, o * P : (o + 1) * P],
                    identity,
                )
            for j in range(3):
                o = og * 3 + j
                nc.vector.tensor_copy(
                    out=r(woT[o][:, jt * P : (jt + 1) * P]),
                    in_=ps[:, j * P : (j + 1) * P],
                )

    # ---- biases ------------------------------------------------------------
    # flat (h e) laid out as [(o p)] -> [p, o]; one chunk = a head pair
    nc.sync.dma_start(out=bq_pairs, in_=bq_d.rearrange("(o h2) e -> (h2 e) o", h2=2))
    nc.sync.dma_start(out=bk_pairs, in_=bk_d.rearrange("(o h2) e -> (h2 e) o", h2=2))
    nc.sync.dma_start(
        out=r(bv_col), in_=r(bv_d.rearrange("(o h2) e -> (h2 e) o", h2=2))
    )
    nc.sync.dma_start(out=bo1, in_=bo_d[None, :])

    # bo' = bo + bv_cat @ Wo^T  (bv folded out of the V matmul)
    ps = sc_psum.tile([P, S], F32, tag="sc", name="sc")
    for o in range(NO):
        nc.tensor.matmul(
            ps[:1, 0:512], r(bv_col[:, o : o + 1]), r(woT[o][:, 0:512]),
            start=(o == 0), stop=False,
        )
        nc.tensor.matmul(
            ps[:1, 512:768], r(bv_col[:, o : o + 1]), r(woT[o][:, 512:768]),
            start=(o == 0), stop=(o == NO - 1),
        )
    nc.vector.tensor_add(out=bo1, in0=ps[:1, 0:D], in1=bo1)
    # broadcast bo' to 128 partitions on the idle GPSIMD engine
    nc.gpsimd.partition_broadcast(boB, bo1, channels=P)

    # ---- V for all heads (lhsT = xT chunk reused across both col groups) ----
    wv_all = singles.tile([P, NO, H, DH], F32, tag="wv_all", name="wv_all")
    for h in range(H):
        nc.sync.dma_start(
            out=r(wv_all[:, :, h, :]),
            in_=r(wv_d[h].rearrange("(o p) e -> p o e", p=P)),
        )
    for t in range(NS if "v" in phases else 0):
        ps = sc_psum.tile([P, S], F32, tag="sc", name="sc")
        for o in range(NO):
            lhsT = r(xT[o][:, t * P : (t + 1) * P])
            nc.tensor.matmul(
                ps[:, 0:512], lhsT, r(wv_all[:, o, 0:8, :]),
                start=(o == 0), stop=False,
            )
            nc.tensor.matmul(
                ps[:, 512:768], lhsT, r(wv_all[:, o, 8:12, :]),
                start=(o == 0), stop=(o == NO - 1),
            )
        nc.scalar.copy(
            out=r(vall[:, t, 0:8, 0:DH]),
            in_=ps[:, 0:512].rearrange("p (h e) -> p h e", e=DH),
        )
        nc.scalar.copy(
            out=r(vall[:, t, 8:12, 0:DH]),
            in_=ps[:, 512:768].rearrange("p (h e) -> p h e", e=DH),
        )

    # ---- attention, one head pair at a time, heads interleaved --------------
    qkpool = pools["qkpool"]
    wpool = pools["wpool"]
    attnpool = pools["attnpool"]
    rpool = pools["rpool"]

    for p in range(NPAIR if "qkv" in phases else 0):
        wq_sb = wpool.tile([P, NO, 2, DH], F32, tag="wq", name="wq")
        wk_sb = wpool.tile([P, NO, 2, DH], F32, tag="wk", name="wk")
        for i in range(2):
            nc.sync.dma_start(
                out=r(wq_sb[:, :, i, :]),
                in_=r(wq_d[2 * p + i].rearrange("(o p) e -> p o e", p=P)),
            )
            nc.sync.dma_start(
                out=r(wk_sb[:, :, i, :]),
                in_=r(wk_d[2 * p + i].rearrange("(o p) e -> p o e", p=P)),
            )

        qt2 = qkpool.tile([P, S], F32, tag="qt2", name="qt2")
        kz = [
            qkpool.tile([P, S], F32, tag=f"kz{i}", name=f"kz{i}") for i in range(2)
        ]
        for which, w_sb, bias in (("q", wq_sb, bq_pairs), ("k", wk_sb, bk_pairs)):
            ps = sc_psum.tile([P, S], F32, tag="sc", name="sc")
            for o in range(NO):
                lhsT = r(w_sb[:, o])
                nc.tensor.matmul(
                    ps[:, 0:512], lhsT, r(xT[o][:, 0:512]),
                    start=(o == 0), stop=False,
                )
                nc.tensor.matmul(
                    ps[:, 512:1024], lhsT, r(xT[o][:, 512:1024]),
                    start=(o == 0), stop=(o == NO - 1),
                )
            if which == "q":
                nc.scalar.activation(
                    out=r(qt2), in_=ps,
                    func=mybir.ActivationFunctionType.Identity,
                    bias=bias[:, p : p + 1],
                )
            else:
                # split K^T into per-head tiles, other head's rows zeroed,
                # so the scores matmul contracts over the full K=128
                for i in range(2):
                    rows = slice(i * DH, (i + 1) * DH)
                    zrows = slice((1 - i) * DH, (2 - i) * DH)
                    nc.vector.tensor_copy(
                        out=r(kz[i][zrows, :]),
                        in_=zeros_f32[0:DH, 0:1].to_broadcast((DH, S)),
                    )
                    nc.scalar.activation(
                        out=r(kz[i][rows, :]), in_=ps[rows, :],
                        func=mybir.ActivationFunctionType.Identity,
                        bias=bias[rows, p : p + 1],
                    )

        if "attn" not in phases:
            continue
        for i in range(2):  # heads serially: frees PSUM banks for depth-3 scores
            h = 2 * p + i
            rows = slice(i * DH, (i + 1) * DH)
            o_ps = out_psum.tile([DH + 1, S], F32, tag="outp", name="outp")
            for t in range(NS):
                s_ps = sc_psum.tile([P, S], F32, tag="sc", name="sc")
                lhsT = r(kz[i][:, t * P : (t + 1) * P])
                nc.tensor.matmul(
                    s_ps[:, 0:512], lhsT, r(qt2[:, 0:512]), start=True, stop=True
                )
                nc.tensor.matmul(
                    s_ps[:, 512:1024], lhsT, r(qt2[:, 512:1024]),
                    start=True, stop=True,
                )
                at = attnpool.tile([P, S], F32, tag="attnT", name="attnT")
                nc.scalar.activation(
                    out=r(at), in_=s_ps,
                    func=mybir.ActivationFunctionType.Exp,
                    scale=float(1.0 / np.sqrt(DH)),
                )
                lhsTv = r(vall[:, t, h, 0 : DH + 1])
                nc.tensor.matmul(
                    o_ps[:, 0:512], lhsTv, r(at[:, 0:512]),
                    start=(t == 0), stop=(t == NS - 1),
                )
                nc.tensor.matmul(
                    o_ps[:, 512:1024], lhsTv, r(at[:, 512:1024]),
                    start=(t == 0), stop=(t == NS - 1),
                )
            recip = rpool.tile([1, S], F32, tag="recip", name="recip")
            nc.vector.reciprocal(out=recip, in_=o_ps[DH : DH + 1, :])
            recipB = rpool.tile([DH, S], F32, tag="recipB", name="recipB")
            nc.gpsimd.partition_broadcast(recipB, recip, channels=DH)
            nc.vector.tensor_mul(
                out=r(outTall[p][rows, :]), in0=o_ps[0:DH, :], in1=recipB
            )

    # ---- output projection (lhsT reused across both n-halves) ---------------
    fpool = pools["fpool"]
    for st in range(NS if "wo" in phases else 0):
        f_ps = sc_psum.tile([P, S], F32, tag="sc", name="sc")
        for o in range(NO):
            lhsT = r(outTall[o][:, st * P : (st + 1) * P])
            nc.tensor.matmul(
                f_ps[:, 0:512], lhsT, r(woT[o][:, 0:512]),
                start=(o == 0), stop=False,
            )
            nc.tensor.matmul(
                f_ps[:, 512:768], lhsT, r(woT[o][:, 512:768]),
                start=(o == 0), stop=(o == NO - 1),
            )
        f_sb = fpool.tile([P, D], F32, tag="f_sb", name="f_sb")
        nc.vector.tensor_add(out=f_sb, in0=f_ps[:, :D], in1=boB)
        nc.sync.dma_start(out=out_d[st * P : (st + 1) * P, :], in_=f_sb)


_NC_CACHE = None


def get_nc():
    global _NC_CACHE
    if _NC_CACHE is None:
        _NC_CACHE = build_nc()
    return _NC_CACHE


def kernel(**inputs):
    nc = get_nc()
    shared = {k: np.ascontiguousarray(np.asarray(inputs[k], dtype=np.float32))
              for k in ("Wq", "bq", "Wk", "bk", "Wv", "bv", "Wo", "bo")}
    x = np.ascontiguousarray(np.asarray(inputs["x"], dtype=np.float32))
    in_maps = [dict(shared, x=x[b]) for b in range(B)]
    res = run_bass_kernel_spmd(nc, in_maps, list(range(N_CORES)))
    out = np.stack([res.results[b]["out"] for b in range(B)], axis=0)
    return out


if __name__ == "__main__":
    nc = build_nc()
    print("build ok")
